# revision 7
# baseline (speedup 1.0000x reference)
"""KANLinear forward on 8 TRN2 NeuronCores, data-parallel over batch.

out = silu(x) @ base_weight.T + einsum('bik,oik->bo', b_splines(x), spline_weight*scaler)

Uniform grid (GRID_SIZE=3, SPLINE_ORDER=3, range [-1,1]): for x in interval
m (knots at -1/3, 1/3), the 6 basis values are bases[m+d] = S_d(t)/6 with
t the in-interval fraction and S_d the standard uniform cubic B-spline
blending polynomials (scaled by 6; the 1/6 is folded into the weights).
"""
import copy
import numpy as np

import concourse.bass as bass
import concourse.mybir as mybir
import concourse.tile as tile
from concourse.vector_clock import ScopedClock
from concourse.bass_utils import run_bass_kernel_spmd

N_CORES = 8
B_FULL, IN_F, OUT_F, NK = 8192, 1024, 1024, 6
B_CORE = B_FULL // N_CORES          # 1024
QB = 256                            # batch block per quarter
NQ = B_CORE // QB                   # 4
NC_IN = IN_F // 128                 # 8 in-chunks
F32, BF16 = mybir.dt.float32, mybir.dt.bfloat16
AF = mybir.ActivationFunctionType

_H = np.float32(2.0) / np.float32(3.0)
G4 = float(np.float32(1.0) * _H + np.float32(-1.0))   # -1/3 knot as the reference computes it
G5 = float(np.float32(2.0) * _H + np.float32(-1.0))   # +1/3 knot

MAX_WAITS = 1
_ws_counter = [0]


def _split_multi_waits(nc):
    """This walrus build rejects >1 sync wait per instruction; move extras
    onto fresh same-engine NoOps inserted just before the instruction."""
    for f in nc.m.functions:
        for bb in f.blocks:
            out, changed = [], False
            for inst in bb.instructions:
                si = getattr(inst, "sync_info", None)
                waits = list(si.on_wait) if si is not None and si.on_wait else []
                if len(waits) > MAX_WAITS:
                    changed = True
                    si.on_wait[:] = waits[-MAX_WAITS:]
                    rest = waits[:-MAX_WAITS]
                    while rest:
                        chunk, rest = rest[:MAX_WAITS], rest[MAX_WAITS:]
                        _ws_counter[0] += 1
                        nop = mybir.InstNoOp(
                            name=f"I-waitsplit-{_ws_counter[0]}", ins=[], outs=[])
                        nop.engine = inst.engine
                        nop.sync_info = mybir.SyncInfo(on_update=[], on_wait=list(chunk))
                        out.append(nop)
                out.append(inst)
            if changed:
                bb.instructions[:] = out


class _TileCtx(tile.TileContext):
    def _drain_and_barrier(self, tick_clock, wait_clock):
        nc = self.nc
        nop0 = nc.sync.nop()
        wait_clock.add_sem_waits(nop0.ins, ScopedClock({None: tick_clock.global_clock}))
        si = nop0.ins.sync_info
        waits = list(si.on_wait) if si and si.on_wait else []
        if len(waits) > MAX_WAITS:
            si.on_wait[:] = waits[:MAX_WAITS]
            rest = waits[MAX_WAITS:]
            while rest:
                chunk, rest = rest[:MAX_WAITS], rest[MAX_WAITS:]
                nopi = nc.sync.nop()
                nsi = nopi.ins.sync_info
                if nsi is None:
                    nopi.ins.sync_info = mybir.SyncInfo(on_update=[], on_wait=list(chunk))
                else:
                    nsi.on_wait[:] = list(chunk)
        nc.sync.drain()
        nc.all_engine_barrier()
        assert self.sems is not None
        popped = nc._tile_sem_poison_stack.pop()
        assert popped is self._sem_poison
        nc.clear_and_free_semaphores(list(self.sems.allocated().values()))
        nc.all_engine_barrier()


def _build():
    nc = bass.Bass(trn_type="TRN2")
    xT = nc.dram_tensor("xT", [IN_F, B_CORE], F32, kind="ExternalInput")
    swT = nc.dram_tensor("swT", [NK, IN_F, OUT_F], F32, kind="ExternalInput")
    ssT = nc.dram_tensor("ssT", [IN_F, OUT_F], F32, kind="ExternalInput")
    bwT = nc.dram_tensor("bwT", [IN_F, OUT_F], F32, kind="ExternalInput")
    Y = nc.dram_tensor("Y", [B_CORE, OUT_F], F32, kind="ExternalOutput")

    with _TileCtx(nc) as tc:
        with tc.tile_pool(name="res", bufs=1) as res, \
             tc.tile_pool(name="xload", bufs=2) as xload, \
             tc.tile_pool(name="qdata", bufs=3) as qdata, \
             tc.tile_pool(name="tmp", bufs=2) as tmp, \
             tc.tile_pool(name="stage", bufs=2) as stage, \
             tc.tile_pool(name="psum", bufs=2, space="PSUM") as psum:

            # ---- const bias APs for ACT affine ops ----
            for cv in (1.5, 1.0, -6.0, 4.0, 2.0):
                cb = res.tile([128, 1], F32, name=f"cb{cv}", tag=f"cb{cv}")
                nc.vector.memset(cb[:], cv)
                nc.const_aps.aps[(F32, cv)] = cb[:]
            # ---- resident weights (spline bf16, base fp32r) ----
            sw_bf = res.tile([128, NK, NC_IN, OUT_F], BF16)   # 96 KB/part
            bw_bf = res.tile([128, NC_IN, OUT_F], mybir.dt.float32r)  # 32 KB/part
            wstream_cm = tc.tile_pool(name="wstream", bufs=1)
            wstream = wstream_cm.__enter__()
            for c in range(NC_IN):
                rows = slice(c * 128, (c + 1) * 128)
                ss_t = wstream.tile([128, OUT_F], F32, tag="ss")
                nc.sync.dma_start(ss_t[:], ssT[rows, :])
                ssr = wstream.tile([128, OUT_F], BF16, tag="ssr")
                nc.vector.tensor_scalar_mul(ssr[:], ss_t[:], 1.0 / 6.0)
                bw_t = wstream.tile([128, OUT_F], F32, tag="bw")
                nc.sync.dma_start(bw_t[:], bwT[rows, :])
                nc.vector.tensor_copy(bw_bf[:, c], bw_t[:])
                for k in range(NK):
                    sw_t = wstream.tile([128, OUT_F], F32, tag="sw")
                    nc.sync.dma_start(sw_t[:], swT[k, rows, :])
                    nc.gpsimd.tensor_tensor(
                        sw_bf[:, k, c], sw_t[:], ssr[:], op=mybir.AluOpType.mult)

            wstream_cm.__exit__(None, None, None)

            # ---- per batch-quarter ----
            for q in range(NQ):
                bsl = slice(q * QB, (q + 1) * QB)
                x_t = xload.tile([128, NC_IN, QB], F32)
                nc.sync.dma_start(
                    x_t[:], xT[:, bsl].rearrange("(c p) b -> p c b", p=128))
                psl = []
                for mb in range(QB // 128):
                    for no in range(OUT_F // 512):
                        ps = psum.tile([128, 512], F32, name=f"ps{mb}{no}", tag=f"ps{mb}{no}")
                        psl.append((mb, no, ps))
                for c in range(NC_IN):
                    xa = x_t[:, c]
                    silu_bf = qdata.tile([128, QB], mybir.dt.float32r, tag="silu", name="silu_bf")
                    bases = qdata.tile([128, NK, QB], BF16, tag="bases", name="bases")
                    nc.scalar.activation(silu_bf[:], xa, AF.Silu)
                    ge1 = tmp.tile([128, QB], BF16, tag="ge1")
                    nc.vector.tensor_scalar(ge1[:], xa, G4, None, op0=mybir.AluOpType.is_ge)
                    ge2 = tmp.tile([128, QB], BF16, tag="ge2")
                    nc.vector.tensor_scalar(ge2[:], xa, G5, None, op0=mybir.AluOpType.is_ge)
                    m_bf = tmp.tile([128, QB], BF16, tag="m")
                    nc.vector.tensor_tensor(m_bf[:], ge1[:], ge2[:], op=mybir.AluOpType.add)
                    u0 = tmp.tile([128, QB], F32, tag="u0")   # u = 1.5x+1.5
                    nc.scalar.activation(u0[:], xa, AF.Identity, bias=1.5, scale=1.5)
                    t = tmp.tile([128, QB], BF16, tag="t")
                    nc.vector.scalar_tensor_tensor(
                        t[:], m_bf[:], -1.0, u0[:],
                        op0=mybir.AluOpType.mult, op1=mybir.AluOpType.add)
                    t2 = tmp.tile([128, QB], BF16, tag="t2")
                    nc.vector.tensor_tensor(t2[:], t[:], t[:], op=mybir.AluOpType.mult)
                    t3 = tmp.tile([128, QB], BF16, tag="t3")
                    nc.vector.tensor_tensor(t3[:], t2[:], t[:], op=mybir.AluOpType.mult)
                    v = tmp.tile([128, QB], BF16, tag="v")
                    nc.scalar.activation(v[:], t[:], AF.Identity, bias=1.0, scale=-1.0)
                    v2 = tmp.tile([128, QB], BF16, tag="v2")
                    nc.vector.tensor_tensor(v2[:], v[:], v[:], op=mybir.AluOpType.mult)
                    S0 = tmp.tile([128, QB], BF16, tag="S0")
                    nc.vector.tensor_tensor(S0[:], v2[:], v[:], op=mybir.AluOpType.mult)
                    w1 = tmp.tile([128, QB], BF16, tag="w1")
                    nc.scalar.activation(w1[:], t[:], AF.Identity, bias=-6.0, scale=3.0)
                    S1s = tmp.tile([128, QB], BF16, tag="S1s")
                    nc.vector.tensor_tensor(S1s[:], w1[:], t2[:], op=mybir.AluOpType.mult)
                    S1 = tmp.tile([128, QB], BF16, tag="S1")
                    nc.scalar.activation(S1[:], S1s[:], AF.Identity, bias=4.0, scale=1.0)
                    sum1 = tmp.tile([128, QB], BF16, tag="sum1")
                    nc.vector.tensor_tensor(sum1[:], S0[:], S1s[:], op=mybir.AluOpType.add)
                    sum2 = tmp.tile([128, QB], BF16, tag="sum2")
                    nc.vector.tensor_tensor(sum2[:], sum1[:], t3[:], op=mybir.AluOpType.add)
                    S2 = tmp.tile([128, QB], BF16, tag="S2")
                    nc.scalar.activation(S2[:], sum2[:], AF.Identity, bias=2.0, scale=-1.0)
                    is0 = tmp.tile([128, QB], BF16, tag="is0")
                    nc.scalar.activation(is0[:], ge1[:], AF.Identity, bias=1.0, scale=-1.0)
                    is1 = tmp.tile([128, QB], BF16, tag="is1")
                    nc.vector.tensor_tensor(is1[:], ge1[:], ge2[:], op=mybir.AluOpType.subtract)
                    is2 = ge2
                    S3 = t3
                    MUL, ADD = mybir.AluOpType.mult, mybir.AluOpType.add
                    TT = nc.vector.tensor_tensor

                    def mk(tag):
                        return tmp.tile([128, QB], BF16, tag=tag, name=tag)
                    TT(bases[:, 0], is0[:], S0[:], op=MUL)
                    a = mk("pa")
                    b = mk("pb")
                    TT(a[:], is0[:], S1[:], op=MUL)
                    TT(b[:], is1[:], S0[:], op=MUL)
                    TT(bases[:, 1], a[:], b[:], op=ADD)
                    a2 = mk("pc")
                    b2 = mk("pd")
                    d2 = mk("pe")
                    TT(a2[:], is0[:], S2[:], op=MUL)
                    TT(b2[:], is1[:], S1[:], op=MUL)
                    TT(d2[:], is2[:], S0[:], op=MUL)
                    e2 = mk("pf")
                    TT(e2[:], a2[:], b2[:], op=ADD)
                    TT(bases[:, 2], e2[:], d2[:], op=ADD)
                    a3 = mk("pg")
                    b3 = mk("ph")
                    d3 = mk("pi")
                    TT(a3[:], is0[:], S3[:], op=MUL)
                    TT(b3[:], is1[:], S2[:], op=MUL)
                    TT(d3[:], is2[:], S1[:], op=MUL)
                    e3 = mk("pj")
                    TT(e3[:], a3[:], b3[:], op=ADD)
                    TT(bases[:, 3], e3[:], d3[:], op=ADD)
                    a4 = mk("pk")
                    b4 = mk("pl")
                    TT(a4[:], is1[:], S3[:], op=MUL)
                    TT(b4[:], is2[:], S2[:], op=MUL)
                    TT(bases[:, 4], a4[:], b4[:], op=ADD)
                    TT(bases[:, 5], is2[:], S3[:], op=MUL)

                    for mb, no, ps in psl:
                        brow = slice(mb * 128, (mb + 1) * 128)
                        ocol = slice(no * 512, (no + 1) * 512)
                        for k in range(NK):
                            nc.tensor.matmul(
                                ps[:], bases[:, k, brow], sw_bf[:, k, c, ocol],
                                start=(c == 0 and k == 0), stop=False)
                        nc.tensor.matmul(
                            ps[:], silu_bf[:, brow], bw_bf[:, c, ocol],
                            start=False, stop=(c == NC_IN - 1))

                for mb, no, ps in psl:
                    ocol = slice(no * 512, (no + 1) * 512)
                    st = stage.tile([128, 512], F32, name="st", tag="st")
                    nc.scalar.copy(st[:], ps[:])
                    nc.sync.dma_start(
                        Y[q * QB + mb * 128: q * QB + (mb + 1) * 128, ocol], st[:])
    _split_multi_waits(nc)
    return nc


_NC_CACHE = None


def kernel(x, base_weight, spline_weight, spline_scaler, grid):
    global _NC_CACHE
    if _NC_CACHE is None:
        _NC_CACHE = _build()
    nc = _NC_CACHE
    swT = np.ascontiguousarray(spline_weight.transpose(2, 1, 0)).astype(np.float32)
    ssT = np.ascontiguousarray(spline_scaler.T).astype(np.float32)
    bwT = np.ascontiguousarray(base_weight.T).astype(np.float32)
    in_maps = []
    for c in range(N_CORES):
        xs = x[c * B_CORE:(c + 1) * B_CORE]
        in_maps.append({
            "xT": np.ascontiguousarray(xs.T).astype(np.float32),
            "swT": swT, "ssT": ssT, "bwT": bwT,
        })
    res = run_bass_kernel_spmd(nc, in_maps, core_ids=list(range(N_CORES)))
    return np.concatenate([res.results[c]["Y"] for c in range(N_CORES)], axis=0)


# revision 11
# speedup vs baseline: 1.1307x; 1.1307x over previous
"""KANLinear forward on 8 TRN2 NeuronCores, data-parallel over batch.

out = silu(x) @ base_weight.T + einsum('bik,oik->bo', b_splines(x), spline_weight*scaler)

Uniform grid (GRID_SIZE=3, SPLINE_ORDER=3, range [-1,1]): for x in interval
m (knots at -1/3, 1/3), the 6 basis values are bases[m+d] = S_d(t)/6 with
t the in-interval fraction and S_d the standard uniform cubic B-spline
blending polynomials (scaled by 6; the 1/6 is folded into the weights).
"""
import copy
import numpy as np

import concourse.bass as bass
import concourse.mybir as mybir
import concourse.tile as tile
from concourse.vector_clock import ScopedClock
from concourse.bass_utils import run_bass_kernel_spmd

N_CORES = 8
B_FULL, IN_F, OUT_F, NK = 8192, 1024, 1024, 6
B_CORE = B_FULL // N_CORES          # 1024
QB = 256                            # batch block per quarter
NQ = B_CORE // QB                   # 4
NC_IN = IN_F // 128                 # 8 in-chunks
F32, BF16 = mybir.dt.float32, mybir.dt.bfloat16
AF = mybir.ActivationFunctionType

_H = np.float32(2.0) / np.float32(3.0)
G4 = float(np.float32(1.0) * _H + np.float32(-1.0))   # -1/3 knot as the reference computes it
G5 = float(np.float32(2.0) * _H + np.float32(-1.0))   # +1/3 knot

MAX_WAITS = 1
_ws_counter = [0]


def _split_multi_waits(nc):
    """This walrus build rejects >1 sync wait per instruction; move extras
    onto fresh same-engine NoOps inserted just before the instruction."""
    for f in nc.m.functions:
        for bb in f.blocks:
            out, changed = [], False
            for inst in bb.instructions:
                si = getattr(inst, "sync_info", None)
                waits = list(si.on_wait) if si is not None and si.on_wait else []
                if len(waits) > MAX_WAITS:
                    changed = True
                    si.on_wait[:] = waits[-MAX_WAITS:]
                    rest = waits[:-MAX_WAITS]
                    while rest:
                        chunk, rest = rest[:MAX_WAITS], rest[MAX_WAITS:]
                        _ws_counter[0] += 1
                        nop = mybir.InstNoOp(
                            name=f"I-waitsplit-{_ws_counter[0]}", ins=[], outs=[])
                        nop.engine = inst.engine
                        nop.sync_info = mybir.SyncInfo(on_update=[], on_wait=list(chunk))
                        out.append(nop)
                out.append(inst)
            if changed:
                bb.instructions[:] = out


class _TileCtx(tile.TileContext):
    def _drain_and_barrier(self, tick_clock, wait_clock):
        nc = self.nc
        nop0 = nc.sync.nop()
        wait_clock.add_sem_waits(nop0.ins, ScopedClock({None: tick_clock.global_clock}))
        si = nop0.ins.sync_info
        waits = list(si.on_wait) if si and si.on_wait else []
        if len(waits) > MAX_WAITS:
            si.on_wait[:] = waits[:MAX_WAITS]
            rest = waits[MAX_WAITS:]
            while rest:
                chunk, rest = rest[:MAX_WAITS], rest[MAX_WAITS:]
                nopi = nc.sync.nop()
                nsi = nopi.ins.sync_info
                if nsi is None:
                    nopi.ins.sync_info = mybir.SyncInfo(on_update=[], on_wait=list(chunk))
                else:
                    nsi.on_wait[:] = list(chunk)
        nc.sync.drain()
        nc.all_engine_barrier()
        assert self.sems is not None
        popped = nc._tile_sem_poison_stack.pop()
        assert popped is self._sem_poison
        nc.clear_and_free_semaphores(list(self.sems.allocated().values()))
        nc.all_engine_barrier()


def _build():
    nc = bass.Bass(trn_type="TRN2")
    xT = nc.dram_tensor("xT", [IN_F, B_CORE], F32, kind="ExternalInput")
    swT = nc.dram_tensor("swT", [NK, IN_F, OUT_F], F32, kind="ExternalInput")
    ssT = nc.dram_tensor("ssT", [IN_F, OUT_F], F32, kind="ExternalInput")
    bwT = nc.dram_tensor("bwT", [IN_F, OUT_F], F32, kind="ExternalInput")
    Y = nc.dram_tensor("Y", [B_CORE, OUT_F], F32, kind="ExternalOutput")

    with _TileCtx(nc) as tc:
        with tc.tile_pool(name="res", bufs=1) as res, \
             tc.tile_pool(name="xload", bufs=2) as xload, \
             tc.tile_pool(name="qdata", bufs=3) as qdata, \
             tc.tile_pool(name="tmp", bufs=2) as tmp, \
             tc.tile_pool(name="stage", bufs=2) as stage, \
             tc.tile_pool(name="psum", bufs=2, space="PSUM") as psum:

            # ---- const bias APs for ACT affine ops ----
            for cv in (1.5, 1.0, -6.0, 4.0, 2.0):
                cb = res.tile([128, 1], F32, name=f"cb{cv}", tag=f"cb{cv}")
                nc.vector.memset(cb[:], cv)
                nc.const_aps.aps[(F32, cv)] = cb[:]
            # ---- resident weights (spline bf16, base fp32r) ----
            sw_bf = res.tile([128, NK, NC_IN, OUT_F], BF16)   # 96 KB/part
            bw_bf = res.tile([128, NC_IN, OUT_F], mybir.dt.float32r)  # 32 KB/part
            wstream_cm = tc.tile_pool(name="wstream", bufs=1)
            wstream = wstream_cm.__enter__()
            for c in range(NC_IN):
                rows = slice(c * 128, (c + 1) * 128)
                ss_t = wstream.tile([128, OUT_F], F32, tag="ss")
                nc.sync.dma_start(ss_t[:], ssT[rows, :])
                ssr = wstream.tile([128, OUT_F], BF16, tag="ssr")
                nc.vector.tensor_scalar_mul(ssr[:], ss_t[:], 1.0 / 6.0)
                bw_t = wstream.tile([128, OUT_F], F32, tag="bw")
                nc.sync.dma_start(bw_t[:], bwT[rows, :])
                nc.vector.tensor_copy(bw_bf[:, c], bw_t[:])
                for k in range(NK):
                    sw_t = wstream.tile([128, OUT_F], F32, tag="sw")
                    nc.sync.dma_start(sw_t[:], swT[k, rows, :])
                    nc.vector.tensor_tensor(
                        sw_bf[:, k, c], sw_t[:], ssr[:], op=mybir.AluOpType.mult)

            wstream_cm.__exit__(None, None, None)

            # ---- per batch-quarter ----
            for q in range(NQ):
                bsl = slice(q * QB, (q + 1) * QB)
                x_t = xload.tile([128, NC_IN, QB], F32)
                nc.sync.dma_start(
                    x_t[:], xT[:, bsl].rearrange("(c p) b -> p c b", p=128))
                psl = []
                for mb in range(QB // 128):
                    for no in range(OUT_F // 512):
                        ps = psum.tile([128, 512], F32, name=f"ps{mb}{no}", tag=f"ps{mb}{no}")
                        psl.append((mb, no, ps))
                for c in range(NC_IN):
                    xa = x_t[:, c]
                    silu_bf = qdata.tile([128, QB], mybir.dt.float32r, tag="silu", name="silu_bf")
                    bases = qdata.tile([128, NK, QB], BF16, tag="bases", name="bases")
                    nc.scalar.activation(silu_bf[:], xa, AF.Silu)
                    ge1 = tmp.tile([128, QB], BF16, tag="ge1")
                    nc.vector.tensor_scalar(ge1[:], xa, G4, None, op0=mybir.AluOpType.is_ge)
                    ge2 = tmp.tile([128, QB], BF16, tag="ge2")
                    nc.vector.tensor_scalar(ge2[:], xa, G5, None, op0=mybir.AluOpType.is_ge)
                    m_bf = tmp.tile([128, QB], BF16, tag="m")
                    nc.vector.tensor_tensor(m_bf[:], ge1[:], ge2[:], op=mybir.AluOpType.add)
                    u0 = tmp.tile([128, QB], F32, tag="u0")   # u = 1.5x+1.5
                    nc.scalar.activation(u0[:], xa, AF.Identity, bias=1.5, scale=1.5)
                    t = tmp.tile([128, QB], BF16, tag="t")
                    nc.vector.scalar_tensor_tensor(
                        t[:], m_bf[:], -1.0, u0[:],
                        op0=mybir.AluOpType.mult, op1=mybir.AluOpType.add)
                    t2 = tmp.tile([128, QB], BF16, tag="t2")
                    nc.vector.tensor_tensor(t2[:], t[:], t[:], op=mybir.AluOpType.mult)
                    t3 = tmp.tile([128, QB], BF16, tag="t3")
                    nc.vector.tensor_tensor(t3[:], t2[:], t[:], op=mybir.AluOpType.mult)
                    v = tmp.tile([128, QB], BF16, tag="v")
                    nc.scalar.activation(v[:], t[:], AF.Identity, bias=1.0, scale=-1.0)
                    v2 = tmp.tile([128, QB], BF16, tag="v2")
                    nc.vector.tensor_tensor(v2[:], v[:], v[:], op=mybir.AluOpType.mult)
                    S0 = tmp.tile([128, QB], BF16, tag="S0")
                    nc.vector.tensor_tensor(S0[:], v2[:], v[:], op=mybir.AluOpType.mult)
                    w1 = tmp.tile([128, QB], BF16, tag="w1")
                    nc.scalar.activation(w1[:], t[:], AF.Identity, bias=-6.0, scale=3.0)
                    S1s = tmp.tile([128, QB], BF16, tag="S1s")
                    nc.vector.tensor_tensor(S1s[:], w1[:], t2[:], op=mybir.AluOpType.mult)
                    S1 = tmp.tile([128, QB], BF16, tag="S1")
                    nc.scalar.activation(S1[:], S1s[:], AF.Identity, bias=4.0, scale=1.0)
                    sum1 = tmp.tile([128, QB], BF16, tag="sum1")
                    nc.vector.tensor_tensor(sum1[:], S0[:], S1s[:], op=mybir.AluOpType.add)
                    sum2 = tmp.tile([128, QB], BF16, tag="sum2")
                    nc.vector.tensor_tensor(sum2[:], sum1[:], t3[:], op=mybir.AluOpType.add)
                    S2 = tmp.tile([128, QB], BF16, tag="S2")
                    nc.scalar.activation(S2[:], sum2[:], AF.Identity, bias=2.0, scale=-1.0)
                    is0 = tmp.tile([128, QB], BF16, tag="is0")
                    nc.scalar.activation(is0[:], ge1[:], AF.Identity, bias=1.0, scale=-1.0)
                    is1 = tmp.tile([128, QB], BF16, tag="is1")
                    nc.vector.tensor_tensor(is1[:], ge1[:], ge2[:], op=mybir.AluOpType.subtract)
                    is2 = ge2
                    S3 = t3
                    MUL, ADD = mybir.AluOpType.mult, mybir.AluOpType.add
                    TT = nc.vector.tensor_tensor

                    def mk(tag):
                        return tmp.tile([128, QB], BF16, tag=tag, name=tag)
                    TT(bases[:, 0], is0[:], S0[:], op=MUL)
                    a = mk("pa")
                    b = mk("pb")
                    TT(a[:], is0[:], S1[:], op=MUL)
                    TT(b[:], is1[:], S0[:], op=MUL)
                    TT(bases[:, 1], a[:], b[:], op=ADD)
                    a2 = mk("pc")
                    b2 = mk("pd")
                    d2 = mk("pe")
                    TT(a2[:], is0[:], S2[:], op=MUL)
                    TT(b2[:], is1[:], S1[:], op=MUL)
                    TT(d2[:], is2[:], S0[:], op=MUL)
                    e2 = mk("pf")
                    TT(e2[:], a2[:], b2[:], op=ADD)
                    TT(bases[:, 2], e2[:], d2[:], op=ADD)
                    a3 = mk("pg")
                    b3 = mk("ph")
                    d3 = mk("pi")
                    TT(a3[:], is0[:], S3[:], op=MUL)
                    TT(b3[:], is1[:], S2[:], op=MUL)
                    TT(d3[:], is2[:], S1[:], op=MUL)
                    e3 = mk("pj")
                    TT(e3[:], a3[:], b3[:], op=ADD)
                    TT(bases[:, 3], e3[:], d3[:], op=ADD)
                    a4 = mk("pk")
                    b4 = mk("pl")
                    TT(a4[:], is1[:], S3[:], op=MUL)
                    TT(b4[:], is2[:], S2[:], op=MUL)
                    TT(bases[:, 4], a4[:], b4[:], op=ADD)
                    TT(bases[:, 5], is2[:], S3[:], op=MUL)

                    for mb, no, ps in psl:
                        brow = slice(mb * 128, (mb + 1) * 128)
                        ocol = slice(no * 512, (no + 1) * 512)
                        for k in range(NK):
                            nc.tensor.matmul(
                                ps[:], bases[:, k, brow], sw_bf[:, k, c, ocol],
                                start=(c == 0 and k == 0), stop=False)
                        nc.tensor.matmul(
                            ps[:], silu_bf[:, brow], bw_bf[:, c, ocol],
                            start=False, stop=(c == NC_IN - 1))

                for mb, no, ps in psl:
                    ocol = slice(no * 512, (no + 1) * 512)
                    st = stage.tile([128, 512], F32, name="st", tag="st")
                    nc.scalar.copy(st[:], ps[:])
                    nc.sync.dma_start(
                        Y[q * QB + mb * 128: q * QB + (mb + 1) * 128, ocol], st[:])
    _split_multi_waits(nc)
    return nc


_NC_CACHE = None


def kernel(x, base_weight, spline_weight, spline_scaler, grid):
    global _NC_CACHE
    if _NC_CACHE is None:
        _NC_CACHE = _build()
    nc = _NC_CACHE
    swT = np.ascontiguousarray(spline_weight.transpose(2, 1, 0)).astype(np.float32)
    ssT = np.ascontiguousarray(spline_scaler.T).astype(np.float32)
    bwT = np.ascontiguousarray(base_weight.T).astype(np.float32)
    in_maps = []
    for c in range(N_CORES):
        xs = x[c * B_CORE:(c + 1) * B_CORE]
        in_maps.append({
            "xT": np.ascontiguousarray(xs.T).astype(np.float32),
            "swT": swT, "ssT": ssT, "bwT": bwT,
        })
    res = run_bass_kernel_spmd(nc, in_maps, core_ids=list(range(N_CORES)))
    return np.concatenate([res.results[c]["Y"] for c in range(N_CORES)], axis=0)


# revision 16
# speedup vs baseline: 1.3045x; 1.1538x over previous
"""KANLinear forward on 8 TRN2 NeuronCores, data-parallel over batch.

out = silu(x) @ base_weight.T + einsum('bik,oik->bo', b_splines(x), spline_weight*scaler)

Uniform grid (GRID_SIZE=3, SPLINE_ORDER=3, range [-1,1]): for x in interval
m (knots at -1/3, 1/3), the 6 basis values are bases[m+d] = S_d(t)/6 with
t the in-interval fraction and S_d the standard uniform cubic B-spline
blending polynomials (scaled by 6; the 1/6 is folded into the weights).
"""
import copy
import numpy as np

import concourse.bass as bass
import concourse.mybir as mybir
import concourse.tile as tile
from concourse.vector_clock import ScopedClock
from concourse.bass_utils import run_bass_kernel_spmd

N_CORES = 8
B_FULL, IN_F, OUT_F, NK = 8192, 1024, 1024, 6
B_CORE = B_FULL // N_CORES          # 1024
QB = 256                            # batch block per quarter
NQ = B_CORE // QB                   # 4
NC_IN = IN_F // 128                 # 8 in-chunks
F32, BF16 = mybir.dt.float32, mybir.dt.bfloat16
AF = mybir.ActivationFunctionType

_H = np.float32(2.0) / np.float32(3.0)
G4 = float(np.float32(1.0) * _H + np.float32(-1.0))   # -1/3 knot as the reference computes it
G5 = float(np.float32(2.0) * _H + np.float32(-1.0))   # +1/3 knot

MAX_WAITS = 1
_ws_counter = [0]


def _split_multi_waits(nc):
    """This walrus build rejects >1 sync wait per instruction; move extras
    onto fresh same-engine NoOps inserted just before the instruction."""
    for f in nc.m.functions:
        for bb in f.blocks:
            out, changed = [], False
            for inst in bb.instructions:
                si = getattr(inst, "sync_info", None)
                waits = list(si.on_wait) if si is not None and si.on_wait else []
                if len(waits) > MAX_WAITS:
                    changed = True
                    si.on_wait[:] = waits[-MAX_WAITS:]
                    rest = waits[:-MAX_WAITS]
                    while rest:
                        chunk, rest = rest[:MAX_WAITS], rest[MAX_WAITS:]
                        _ws_counter[0] += 1
                        nop = mybir.InstNoOp(
                            name=f"I-waitsplit-{_ws_counter[0]}", ins=[], outs=[])
                        nop.engine = inst.engine
                        nop.sync_info = mybir.SyncInfo(on_update=[], on_wait=list(chunk))
                        out.append(nop)
                out.append(inst)
            if changed:
                bb.instructions[:] = out


class _TileCtx(tile.TileContext):
    def _drain_and_barrier(self, tick_clock, wait_clock):
        nc = self.nc
        nop0 = nc.sync.nop()
        wait_clock.add_sem_waits(nop0.ins, ScopedClock({None: tick_clock.global_clock}))
        si = nop0.ins.sync_info
        waits = list(si.on_wait) if si and si.on_wait else []
        if len(waits) > MAX_WAITS:
            si.on_wait[:] = waits[:MAX_WAITS]
            rest = waits[MAX_WAITS:]
            while rest:
                chunk, rest = rest[:MAX_WAITS], rest[MAX_WAITS:]
                nopi = nc.sync.nop()
                nsi = nopi.ins.sync_info
                if nsi is None:
                    nopi.ins.sync_info = mybir.SyncInfo(on_update=[], on_wait=list(chunk))
                else:
                    nsi.on_wait[:] = list(chunk)
        nc.sync.drain()
        nc.all_engine_barrier()
        assert self.sems is not None
        popped = nc._tile_sem_poison_stack.pop()
        assert popped is self._sem_poison
        nc.clear_and_free_semaphores(list(self.sems.allocated().values()))
        nc.all_engine_barrier()


def _build():
    nc = bass.Bass(trn_type="TRN2")
    xT = nc.dram_tensor("xT", [IN_F, B_CORE], F32, kind="ExternalInput")
    swT = nc.dram_tensor("swT", [NK, IN_F, OUT_F], F32, kind="ExternalInput")
    ssT = nc.dram_tensor("ssT", [IN_F, OUT_F], F32, kind="ExternalInput")
    bwT = nc.dram_tensor("bwT", [IN_F, OUT_F], F32, kind="ExternalInput")
    Y = nc.dram_tensor("Y", [B_CORE, OUT_F], F32, kind="ExternalOutput")

    with _TileCtx(nc) as tc:
        with tc.tile_pool(name="res", bufs=1) as res, \
             tc.tile_pool(name="xload", bufs=1) as xload, \
             tc.tile_pool(name="qdata", bufs=3) as qdata, \
             tc.tile_pool(name="tmp", bufs=2) as tmp, \
             tc.tile_pool(name="stage", bufs=2) as stage, \
             tc.tile_pool(name="psum", bufs=2, space="PSUM") as psum:

            # ---- const bias APs for ACT affine ops ----
            for cv in (1.5, 1.0, -6.0, 4.0, 2.0):
                cb = res.tile([128, 1], F32, name=f"cb{cv}", tag=f"cb{cv}")
                nc.vector.memset(cb[:], cv)
                nc.const_aps.aps[(F32, cv)] = cb[:]
            # ---- resident weights (spline bf16, base fp32r) ----
            sw_bf = res.tile([128, NK, NC_IN, OUT_F], BF16)   # 96 KB/part
            bw_bf = res.tile([128, NC_IN, OUT_F], mybir.dt.float32r)  # 32 KB/part
            wstream_cm = tc.tile_pool(name="wstream", bufs=1)
            wstream = wstream_cm.__enter__()
            for c in range(NC_IN):
                rows = slice(c * 128, (c + 1) * 128)
                ss_t = wstream.tile([128, OUT_F], F32, tag="ss")
                nc.sync.dma_start(ss_t[:], ssT[rows, :])
                ssr = wstream.tile([128, OUT_F], BF16, tag="ssr")
                nc.vector.tensor_scalar_mul(ssr[:], ss_t[:], 1.0 / 6.0)
                bw_t = wstream.tile([128, OUT_F], F32, tag="bw")
                nc.sync.dma_start(bw_t[:], bwT[rows, :])
                nc.vector.tensor_copy(bw_bf[:, c], bw_t[:])
                for k in range(NK):
                    sw_t = wstream.tile([128, OUT_F], F32, tag="sw", bufs=3)
                    nc.sync.dma_start(sw_t[:], swT[k, rows, :])
                    nc.vector.tensor_tensor(
                        sw_bf[:, k, c], sw_t[:], ssr[:], op=mybir.AluOpType.mult)

            wstream_cm.__exit__(None, None, None)

            # ---- per batch-quarter ----
            for q in range(NQ):
                bsl = slice(q * QB, (q + 1) * QB)
                x_t = xload.tile([128, NC_IN, QB], F32)
                nc.sync.dma_start(
                    x_t[:], xT[:, bsl].rearrange("(c p) b -> p c b", p=128))
                psl = []
                for mb in range(QB // 128):
                    for no in range(OUT_F // 512):
                        ps = psum.tile([128, 512], F32, name=f"ps{mb}{no}", tag=f"ps{mb}{no}")
                        psl.append((mb, no, ps))
                for c in range(NC_IN):
                    xa = x_t[:, c]
                    silu_bf = qdata.tile([128, QB], mybir.dt.float32r, tag="silu", name="silu_bf")
                    bases = qdata.tile([128, NK, QB], BF16, tag="bases", name="bases")
                    nc.scalar.activation(silu_bf[:], xa, AF.Silu)
                    ge1 = tmp.tile([128, QB], BF16, tag="ge1")
                    nc.vector.tensor_scalar(ge1[:], xa, G4, None, op0=mybir.AluOpType.is_ge)
                    ge2 = tmp.tile([128, QB], BF16, tag="ge2")
                    nc.vector.tensor_scalar(ge2[:], xa, G5, None, op0=mybir.AluOpType.is_ge)
                    m_bf = tmp.tile([128, QB], BF16, tag="m")
                    nc.vector.tensor_tensor(m_bf[:], ge1[:], ge2[:], op=mybir.AluOpType.add)
                    u0 = tmp.tile([128, QB], F32, tag="u0")   # u = 1.5x+1.5
                    nc.scalar.activation(u0[:], xa, AF.Identity, bias=1.5, scale=1.5)
                    t = tmp.tile([128, QB], BF16, tag="t")
                    nc.vector.scalar_tensor_tensor(
                        t[:], m_bf[:], -1.0, u0[:],
                        op0=mybir.AluOpType.mult, op1=mybir.AluOpType.add)
                    t2 = tmp.tile([128, QB], BF16, tag="t2")
                    nc.vector.tensor_tensor(t2[:], t[:], t[:], op=mybir.AluOpType.mult)
                    t3 = tmp.tile([128, QB], BF16, tag="t3")
                    nc.vector.tensor_tensor(t3[:], t2[:], t[:], op=mybir.AluOpType.mult)
                    v = tmp.tile([128, QB], BF16, tag="v")
                    nc.scalar.activation(v[:], t[:], AF.Identity, bias=1.0, scale=-1.0)
                    v2 = tmp.tile([128, QB], BF16, tag="v2")
                    nc.vector.tensor_tensor(v2[:], v[:], v[:], op=mybir.AluOpType.mult)
                    S0 = tmp.tile([128, QB], BF16, tag="S0")
                    nc.vector.tensor_tensor(S0[:], v2[:], v[:], op=mybir.AluOpType.mult)
                    w1 = tmp.tile([128, QB], BF16, tag="w1")
                    nc.scalar.activation(w1[:], t[:], AF.Identity, bias=-6.0, scale=3.0)
                    S1s = tmp.tile([128, QB], BF16, tag="S1s")
                    nc.vector.tensor_tensor(S1s[:], w1[:], t2[:], op=mybir.AluOpType.mult)
                    S1 = tmp.tile([128, QB], BF16, tag="S1")
                    nc.scalar.activation(S1[:], S1s[:], AF.Identity, bias=4.0, scale=1.0)
                    sum1 = tmp.tile([128, QB], BF16, tag="sum1")
                    nc.vector.tensor_tensor(sum1[:], S0[:], S1s[:], op=mybir.AluOpType.add)
                    sum2 = tmp.tile([128, QB], BF16, tag="sum2")
                    nc.vector.tensor_tensor(sum2[:], sum1[:], t3[:], op=mybir.AluOpType.add)
                    S2 = tmp.tile([128, QB], BF16, tag="S2")
                    nc.scalar.activation(S2[:], sum2[:], AF.Identity, bias=2.0, scale=-1.0)
                    is0 = tmp.tile([128, QB], BF16, tag="is0")
                    nc.scalar.activation(is0[:], ge1[:], AF.Identity, bias=1.0, scale=-1.0)
                    is1 = tmp.tile([128, QB], BF16, tag="is1")
                    nc.vector.tensor_tensor(is1[:], ge1[:], ge2[:], op=mybir.AluOpType.subtract)
                    is2 = ge2
                    S3 = t3
                    MUL, ADD = mybir.AluOpType.mult, mybir.AluOpType.add
                    TT = nc.vector.tensor_tensor

                    def mk(tag):
                        return tmp.tile([128, QB], BF16, tag=tag, name=tag)
                    TT(bases[:, 0], is0[:], S0[:], op=MUL)
                    a = mk("pa")
                    b = mk("pb")
                    TT(a[:], is0[:], S1[:], op=MUL)
                    TT(b[:], is1[:], S0[:], op=MUL)
                    TT(bases[:, 1], a[:], b[:], op=ADD)
                    a2 = mk("pc")
                    b2 = mk("pd")
                    d2 = mk("pe")
                    TT(a2[:], is0[:], S2[:], op=MUL)
                    TT(b2[:], is1[:], S1[:], op=MUL)
                    TT(d2[:], is2[:], S0[:], op=MUL)
                    e2 = mk("pf")
                    TT(e2[:], a2[:], b2[:], op=ADD)
                    TT(bases[:, 2], e2[:], d2[:], op=ADD)
                    a3 = mk("pg")
                    b3 = mk("ph")
                    d3 = mk("pi")
                    TT(a3[:], is0[:], S3[:], op=MUL)
                    TT(b3[:], is1[:], S2[:], op=MUL)
                    TT(d3[:], is2[:], S1[:], op=MUL)
                    e3 = mk("pj")
                    TT(e3[:], a3[:], b3[:], op=ADD)
                    TT(bases[:, 3], e3[:], d3[:], op=ADD)
                    a4 = mk("pk")
                    b4 = mk("pl")
                    TT(a4[:], is1[:], S3[:], op=MUL)
                    TT(b4[:], is2[:], S2[:], op=MUL)
                    TT(bases[:, 4], a4[:], b4[:], op=ADD)
                    TT(bases[:, 5], is2[:], S3[:], op=MUL)

                    for mb, no, ps in psl:
                        brow = slice(mb * 128, (mb + 1) * 128)
                        ocol = slice(no * 512, (no + 1) * 512)
                        for k in range(NK):
                            nc.tensor.matmul(
                                ps[:], bases[:, k, brow], sw_bf[:, k, c, ocol],
                                start=(c == 0 and k == 0), stop=False)
                        nc.tensor.matmul(
                            ps[:], silu_bf[:, brow], bw_bf[:, c, ocol],
                            start=False, stop=(c == NC_IN - 1))

                for mb, no, ps in psl:
                    ocol = slice(no * 512, (no + 1) * 512)
                    st = stage.tile([128, 512], F32, name="st", tag="st")
                    nc.scalar.copy(st[:], ps[:])
                    nc.sync.dma_start(
                        Y[q * QB + mb * 128: q * QB + (mb + 1) * 128, ocol], st[:])
    _split_multi_waits(nc)
    return nc


_NC_CACHE = None


def kernel(x, base_weight, spline_weight, spline_scaler, grid):
    global _NC_CACHE
    if _NC_CACHE is None:
        _NC_CACHE = _build()
    nc = _NC_CACHE
    swT = np.ascontiguousarray(spline_weight.transpose(2, 1, 0)).astype(np.float32)
    ssT = np.ascontiguousarray(spline_scaler.T).astype(np.float32)
    bwT = np.ascontiguousarray(base_weight.T).astype(np.float32)
    in_maps = []
    for c in range(N_CORES):
        xs = x[c * B_CORE:(c + 1) * B_CORE]
        in_maps.append({
            "xT": np.ascontiguousarray(xs.T).astype(np.float32),
            "swT": swT, "ssT": ssT, "bwT": bwT,
        })
    res = run_bass_kernel_spmd(nc, in_maps, core_ids=list(range(N_CORES)))
    return np.concatenate([res.results[c]["Y"] for c in range(N_CORES)], axis=0)


# revision 17
# speedup vs baseline: 1.5009x; 1.1505x over previous
"""KANLinear forward on 8 TRN2 NeuronCores, data-parallel over batch.

out = silu(x) @ base_weight.T + einsum('bik,oik->bo', b_splines(x), spline_weight*scaler)

Uniform grid (GRID_SIZE=3, SPLINE_ORDER=3, range [-1,1]): for x in interval
m (knots at -1/3, 1/3), the 6 basis values are bases[m+d] = S_d(t)/6 with
t the in-interval fraction and S_d the standard uniform cubic B-spline
blending polynomials (scaled by 6; the 1/6 is folded into the weights).
"""
import copy
import numpy as np

import concourse.bass as bass
import concourse.mybir as mybir
import concourse.tile as tile
from concourse.vector_clock import ScopedClock
from concourse.bass_utils import run_bass_kernel_spmd

N_CORES = 8
B_FULL, IN_F, OUT_F, NK = 8192, 1024, 1024, 6
B_CORE = B_FULL // N_CORES          # 1024
QB = 256                            # batch block per quarter
NQ = B_CORE // QB                   # 4
NC_IN = IN_F // 128                 # 8 in-chunks
F32, BF16 = mybir.dt.float32, mybir.dt.bfloat16
AF = mybir.ActivationFunctionType

_H = np.float32(2.0) / np.float32(3.0)
G4 = float(np.float32(1.0) * _H + np.float32(-1.0))   # -1/3 knot as the reference computes it
G5 = float(np.float32(2.0) * _H + np.float32(-1.0))   # +1/3 knot

MAX_WAITS = 1
_ws_counter = [0]


def _split_multi_waits(nc):
    """This walrus build rejects >1 sync wait per instruction; move extras
    onto fresh same-engine NoOps inserted just before the instruction."""
    for f in nc.m.functions:
        for bb in f.blocks:
            out, changed = [], False
            for inst in bb.instructions:
                si = getattr(inst, "sync_info", None)
                waits = list(si.on_wait) if si is not None and si.on_wait else []
                if len(waits) > MAX_WAITS:
                    changed = True
                    si.on_wait[:] = waits[-MAX_WAITS:]
                    rest = waits[:-MAX_WAITS]
                    while rest:
                        chunk, rest = rest[:MAX_WAITS], rest[MAX_WAITS:]
                        _ws_counter[0] += 1
                        nop = mybir.InstNoOp(
                            name=f"I-waitsplit-{_ws_counter[0]}", ins=[], outs=[])
                        nop.engine = inst.engine
                        nop.sync_info = mybir.SyncInfo(on_update=[], on_wait=list(chunk))
                        out.append(nop)
                out.append(inst)
            if changed:
                bb.instructions[:] = out


class _TileCtx(tile.TileContext):
    def _drain_and_barrier(self, tick_clock, wait_clock):
        nc = self.nc
        nop0 = nc.sync.nop()
        wait_clock.add_sem_waits(nop0.ins, ScopedClock({None: tick_clock.global_clock}))
        si = nop0.ins.sync_info
        waits = list(si.on_wait) if si and si.on_wait else []
        if len(waits) > MAX_WAITS:
            si.on_wait[:] = waits[:MAX_WAITS]
            rest = waits[MAX_WAITS:]
            while rest:
                chunk, rest = rest[:MAX_WAITS], rest[MAX_WAITS:]
                nopi = nc.sync.nop()
                nsi = nopi.ins.sync_info
                if nsi is None:
                    nopi.ins.sync_info = mybir.SyncInfo(on_update=[], on_wait=list(chunk))
                else:
                    nsi.on_wait[:] = list(chunk)
        nc.sync.drain()
        nc.all_engine_barrier()
        assert self.sems is not None
        popped = nc._tile_sem_poison_stack.pop()
        assert popped is self._sem_poison
        nc.clear_and_free_semaphores(list(self.sems.allocated().values()))
        nc.all_engine_barrier()


def _build():
    nc = bass.Bass(trn_type="TRN2")
    xT = nc.dram_tensor("xT", [IN_F, B_CORE], F32, kind="ExternalInput")
    swT = nc.dram_tensor("swT", [NK, IN_F, OUT_F], BF16, kind="ExternalInput")
    ssT = nc.dram_tensor("ssT", [IN_F, OUT_F], F32, kind="ExternalInput")
    bwT = nc.dram_tensor("bwT", [IN_F, OUT_F], F32, kind="ExternalInput")
    Y = nc.dram_tensor("Y", [B_CORE, OUT_F], F32, kind="ExternalOutput")

    with _TileCtx(nc) as tc:
        with tc.tile_pool(name="res", bufs=1) as res, \
             tc.tile_pool(name="xload", bufs=1) as xload, \
             tc.tile_pool(name="qdata", bufs=3) as qdata, \
             tc.tile_pool(name="tmp", bufs=2) as tmp, \
             tc.tile_pool(name="stage", bufs=2) as stage, \
             tc.tile_pool(name="psum", bufs=2, space="PSUM") as psum:

            # ---- const bias APs for ACT affine ops ----
            for cv in (1.5, 1.0, -6.0, 4.0, 2.0):
                cb = res.tile([128, 1], F32, name=f"cb{cv}", tag=f"cb{cv}")
                nc.vector.memset(cb[:], cv)
                nc.const_aps.aps[(F32, cv)] = cb[:]
            # ---- resident weights (spline bf16, base fp32r) ----
            sw_bf = res.tile([128, NK, NC_IN, OUT_F], BF16)   # 96 KB/part
            bw_bf = res.tile([128, NC_IN, OUT_F], mybir.dt.float32r)  # 32 KB/part
            wstream_cm = tc.tile_pool(name="wstream", bufs=1)
            wstream = wstream_cm.__enter__()
            for c in range(NC_IN):
                rows = slice(c * 128, (c + 1) * 128)
                ss_t = wstream.tile([128, OUT_F], F32, tag="ss")
                nc.sync.dma_start(ss_t[:], ssT[rows, :])
                ssr = wstream.tile([128, OUT_F], BF16, tag="ssr")
                nc.vector.tensor_scalar_mul(ssr[:], ss_t[:], 1.0 / 6.0)
                bw_t = wstream.tile([128, OUT_F], F32, tag="bw")
                nc.sync.dma_start(bw_t[:], bwT[rows, :])
                nc.vector.tensor_copy(bw_bf[:, c], bw_t[:])
                for k in range(NK):
                    sw_t = wstream.tile([128, OUT_F], BF16, tag="sw", bufs=3)
                    nc.sync.dma_start(sw_t[:], swT[k, rows, :])
                    nc.vector.tensor_tensor(
                        sw_bf[:, k, c], sw_t[:], ssr[:], op=mybir.AluOpType.mult)

            wstream_cm.__exit__(None, None, None)

            # ---- per batch-quarter ----
            for q in range(NQ):
                bsl = slice(q * QB, (q + 1) * QB)
                x_t = xload.tile([128, NC_IN, QB], F32)
                nc.sync.dma_start(
                    x_t[:], xT[:, bsl].rearrange("(c p) b -> p c b", p=128))
                psl = []
                for mb in range(QB // 128):
                    for no in range(OUT_F // 512):
                        ps = psum.tile([128, 512], F32, name=f"ps{mb}{no}", tag=f"ps{mb}{no}")
                        psl.append((mb, no, ps))
                for c in range(NC_IN):
                    xa = x_t[:, c]
                    silu_bf = qdata.tile([128, QB], mybir.dt.float32r, tag="silu", name="silu_bf")
                    bases = qdata.tile([128, NK, QB], BF16, tag="bases", name="bases")
                    nc.scalar.activation(silu_bf[:], xa, AF.Silu)
                    ge1 = tmp.tile([128, QB], BF16, tag="ge1")
                    nc.vector.tensor_scalar(ge1[:], xa, G4, None, op0=mybir.AluOpType.is_ge)
                    ge2 = tmp.tile([128, QB], BF16, tag="ge2")
                    nc.vector.tensor_scalar(ge2[:], xa, G5, None, op0=mybir.AluOpType.is_ge)
                    m_bf = tmp.tile([128, QB], BF16, tag="m")
                    nc.vector.tensor_tensor(m_bf[:], ge1[:], ge2[:], op=mybir.AluOpType.add)
                    u0 = tmp.tile([128, QB], F32, tag="u0")   # u = 1.5x+1.5
                    nc.scalar.activation(u0[:], xa, AF.Identity, bias=1.5, scale=1.5)
                    t = tmp.tile([128, QB], BF16, tag="t")
                    nc.vector.scalar_tensor_tensor(
                        t[:], m_bf[:], -1.0, u0[:],
                        op0=mybir.AluOpType.mult, op1=mybir.AluOpType.add)
                    t2 = tmp.tile([128, QB], BF16, tag="t2")
                    nc.vector.tensor_tensor(t2[:], t[:], t[:], op=mybir.AluOpType.mult)
                    t3 = tmp.tile([128, QB], BF16, tag="t3")
                    nc.vector.tensor_tensor(t3[:], t2[:], t[:], op=mybir.AluOpType.mult)
                    v = tmp.tile([128, QB], BF16, tag="v")
                    nc.scalar.activation(v[:], t[:], AF.Identity, bias=1.0, scale=-1.0)
                    v2 = tmp.tile([128, QB], BF16, tag="v2")
                    nc.vector.tensor_tensor(v2[:], v[:], v[:], op=mybir.AluOpType.mult)
                    S0 = tmp.tile([128, QB], BF16, tag="S0")
                    nc.vector.tensor_tensor(S0[:], v2[:], v[:], op=mybir.AluOpType.mult)
                    w1 = tmp.tile([128, QB], BF16, tag="w1")
                    nc.scalar.activation(w1[:], t[:], AF.Identity, bias=-6.0, scale=3.0)
                    S1s = tmp.tile([128, QB], BF16, tag="S1s")
                    nc.vector.tensor_tensor(S1s[:], w1[:], t2[:], op=mybir.AluOpType.mult)
                    S1 = tmp.tile([128, QB], BF16, tag="S1")
                    nc.scalar.activation(S1[:], S1s[:], AF.Identity, bias=4.0, scale=1.0)
                    sum1 = tmp.tile([128, QB], BF16, tag="sum1")
                    nc.vector.tensor_tensor(sum1[:], S0[:], S1s[:], op=mybir.AluOpType.add)
                    sum2 = tmp.tile([128, QB], BF16, tag="sum2")
                    nc.vector.tensor_tensor(sum2[:], sum1[:], t3[:], op=mybir.AluOpType.add)
                    S2 = tmp.tile([128, QB], BF16, tag="S2")
                    nc.scalar.activation(S2[:], sum2[:], AF.Identity, bias=2.0, scale=-1.0)
                    is0 = tmp.tile([128, QB], BF16, tag="is0")
                    nc.scalar.activation(is0[:], ge1[:], AF.Identity, bias=1.0, scale=-1.0)
                    is1 = tmp.tile([128, QB], BF16, tag="is1")
                    nc.vector.tensor_tensor(is1[:], ge1[:], ge2[:], op=mybir.AluOpType.subtract)
                    is2 = ge2
                    S3 = t3
                    MUL, ADD = mybir.AluOpType.mult, mybir.AluOpType.add
                    TT = nc.vector.tensor_tensor

                    def mk(tag):
                        return tmp.tile([128, QB], BF16, tag=tag, name=tag)
                    TT(bases[:, 0], is0[:], S0[:], op=MUL)
                    a = mk("pa")
                    b = mk("pb")
                    TT(a[:], is0[:], S1[:], op=MUL)
                    TT(b[:], is1[:], S0[:], op=MUL)
                    TT(bases[:, 1], a[:], b[:], op=ADD)
                    a2 = mk("pc")
                    b2 = mk("pd")
                    d2 = mk("pe")
                    TT(a2[:], is0[:], S2[:], op=MUL)
                    TT(b2[:], is1[:], S1[:], op=MUL)
                    TT(d2[:], is2[:], S0[:], op=MUL)
                    e2 = mk("pf")
                    TT(e2[:], a2[:], b2[:], op=ADD)
                    TT(bases[:, 2], e2[:], d2[:], op=ADD)
                    a3 = mk("pg")
                    b3 = mk("ph")
                    d3 = mk("pi")
                    TT(a3[:], is0[:], S3[:], op=MUL)
                    TT(b3[:], is1[:], S2[:], op=MUL)
                    TT(d3[:], is2[:], S1[:], op=MUL)
                    e3 = mk("pj")
                    TT(e3[:], a3[:], b3[:], op=ADD)
                    TT(bases[:, 3], e3[:], d3[:], op=ADD)
                    a4 = mk("pk")
                    b4 = mk("pl")
                    TT(a4[:], is1[:], S3[:], op=MUL)
                    TT(b4[:], is2[:], S2[:], op=MUL)
                    TT(bases[:, 4], a4[:], b4[:], op=ADD)
                    TT(bases[:, 5], is2[:], S3[:], op=MUL)

                    for mb, no, ps in psl:
                        brow = slice(mb * 128, (mb + 1) * 128)
                        ocol = slice(no * 512, (no + 1) * 512)
                        for k in range(NK):
                            nc.tensor.matmul(
                                ps[:], bases[:, k, brow], sw_bf[:, k, c, ocol],
                                start=(c == 0 and k == 0), stop=False)
                        nc.tensor.matmul(
                            ps[:], silu_bf[:, brow], bw_bf[:, c, ocol],
                            start=False, stop=(c == NC_IN - 1))

                for mb, no, ps in psl:
                    ocol = slice(no * 512, (no + 1) * 512)
                    st = stage.tile([128, 512], F32, name="st", tag="st")
                    nc.scalar.copy(st[:], ps[:])
                    nc.sync.dma_start(
                        Y[q * QB + mb * 128: q * QB + (mb + 1) * 128, ocol], st[:])
    _split_multi_waits(nc)
    return nc


_NC_CACHE = None


def kernel(x, base_weight, spline_weight, spline_scaler, grid):
    global _NC_CACHE
    if _NC_CACHE is None:
        _NC_CACHE = _build()
    nc = _NC_CACHE
    import ml_dtypes
    swT = np.ascontiguousarray(spline_weight.transpose(2, 1, 0)).astype(ml_dtypes.bfloat16)
    ssT = np.ascontiguousarray(spline_scaler.T).astype(np.float32)
    bwT = np.ascontiguousarray(base_weight.T).astype(np.float32)
    in_maps = []
    for c in range(N_CORES):
        xs = x[c * B_CORE:(c + 1) * B_CORE]
        in_maps.append({
            "xT": np.ascontiguousarray(xs.T).astype(np.float32),
            "swT": swT, "ssT": ssT, "bwT": bwT,
        })
    res = run_bass_kernel_spmd(nc, in_maps, core_ids=list(range(N_CORES)))
    return np.concatenate([res.results[c]["Y"] for c in range(N_CORES)], axis=0)


# revision 18
# speedup vs baseline: 1.5037x; 1.0019x over previous
"""KANLinear forward on 8 TRN2 NeuronCores, data-parallel over batch.

out = silu(x) @ base_weight.T + einsum('bik,oik->bo', b_splines(x), spline_weight*scaler)

Uniform grid (GRID_SIZE=3, SPLINE_ORDER=3, range [-1,1]): for x in interval
m (knots at -1/3, 1/3), the 6 basis values are bases[m+d] = S_d(t)/6 with
t the in-interval fraction and S_d the standard uniform cubic B-spline
blending polynomials (scaled by 6; the 1/6 is folded into the weights).
"""
import copy
import numpy as np

import concourse.bass as bass
import concourse.mybir as mybir
import concourse.tile as tile
from concourse.vector_clock import ScopedClock
from concourse.bass_utils import run_bass_kernel_spmd

N_CORES = 8
B_FULL, IN_F, OUT_F, NK = 8192, 1024, 1024, 6
B_CORE = B_FULL // N_CORES          # 1024
QB = 256                            # batch block per quarter
NQ = B_CORE // QB                   # 4
NC_IN = IN_F // 128                 # 8 in-chunks
F32, BF16 = mybir.dt.float32, mybir.dt.bfloat16
AF = mybir.ActivationFunctionType

_H = np.float32(2.0) / np.float32(3.0)
G4 = float(np.float32(1.0) * _H + np.float32(-1.0))   # -1/3 knot as the reference computes it
G5 = float(np.float32(2.0) * _H + np.float32(-1.0))   # +1/3 knot

MAX_WAITS = 1
_ws_counter = [0]


def _split_multi_waits(nc):
    """This walrus build rejects >1 sync wait per instruction; move extras
    onto fresh same-engine NoOps inserted just before the instruction."""
    for f in nc.m.functions:
        for bb in f.blocks:
            out, changed = [], False
            for inst in bb.instructions:
                si = getattr(inst, "sync_info", None)
                waits = list(si.on_wait) if si is not None and si.on_wait else []
                if len(waits) > MAX_WAITS:
                    changed = True
                    si.on_wait[:] = waits[-MAX_WAITS:]
                    rest = waits[:-MAX_WAITS]
                    while rest:
                        chunk, rest = rest[:MAX_WAITS], rest[MAX_WAITS:]
                        _ws_counter[0] += 1
                        nop = mybir.InstNoOp(
                            name=f"I-waitsplit-{_ws_counter[0]}", ins=[], outs=[])
                        nop.engine = inst.engine
                        nop.sync_info = mybir.SyncInfo(on_update=[], on_wait=list(chunk))
                        out.append(nop)
                out.append(inst)
            if changed:
                bb.instructions[:] = out


class _TileCtx(tile.TileContext):
    def _drain_and_barrier(self, tick_clock, wait_clock):
        nc = self.nc
        nop0 = nc.sync.nop()
        wait_clock.add_sem_waits(nop0.ins, ScopedClock({None: tick_clock.global_clock}))
        si = nop0.ins.sync_info
        waits = list(si.on_wait) if si and si.on_wait else []
        if len(waits) > MAX_WAITS:
            si.on_wait[:] = waits[:MAX_WAITS]
            rest = waits[MAX_WAITS:]
            while rest:
                chunk, rest = rest[:MAX_WAITS], rest[MAX_WAITS:]
                nopi = nc.sync.nop()
                nsi = nopi.ins.sync_info
                if nsi is None:
                    nopi.ins.sync_info = mybir.SyncInfo(on_update=[], on_wait=list(chunk))
                else:
                    nsi.on_wait[:] = list(chunk)
        nc.sync.drain()
        nc.all_engine_barrier()
        assert self.sems is not None
        popped = nc._tile_sem_poison_stack.pop()
        assert popped is self._sem_poison
        nc.clear_and_free_semaphores(list(self.sems.allocated().values()))
        nc.all_engine_barrier()


def _build():
    nc = bass.Bass(trn_type="TRN2")
    xT = nc.dram_tensor("xT", [IN_F, B_CORE], F32, kind="ExternalInput")
    swT = nc.dram_tensor("swT", [NK, IN_F, OUT_F], BF16, kind="ExternalInput")
    ssT = nc.dram_tensor("ssT", [IN_F, OUT_F], F32, kind="ExternalInput")
    bwT = nc.dram_tensor("bwT", [IN_F, OUT_F], F32, kind="ExternalInput")
    Y = nc.dram_tensor("Y", [B_CORE, OUT_F], F32, kind="ExternalOutput")

    with _TileCtx(nc) as tc:
        with tc.tile_pool(name="res", bufs=1) as res, \
             tc.tile_pool(name="xload", bufs=2) as xload, \
             tc.tile_pool(name="qdata", bufs=3) as qdata, \
             tc.tile_pool(name="tmp", bufs=2) as tmp, \
             tc.tile_pool(name="stage", bufs=2) as stage, \
             tc.tile_pool(name="psum", bufs=2, space="PSUM") as psum:

            # ---- const bias APs for ACT affine ops ----
            for cv in (1.5, 1.0, -6.0, 4.0, 2.0):
                cb = res.tile([128, 1], F32, name=f"cb{cv}", tag=f"cb{cv}")
                nc.vector.memset(cb[:], cv)
                nc.const_aps.aps[(F32, cv)] = cb[:]
            # ---- resident weights (spline bf16, base fp32r) ----
            sw_bf = res.tile([128, NK, NC_IN, OUT_F], BF16)   # 96 KB/part
            bw_bf = res.tile([128, NC_IN, OUT_F], mybir.dt.float32r)  # 32 KB/part
            wstream_cm = tc.tile_pool(name="wstream", bufs=1)
            wstream = wstream_cm.__enter__()
            for c in range(NC_IN):
                rows = slice(c * 128, (c + 1) * 128)
                ss_t = wstream.tile([128, OUT_F], F32, tag="ss")
                nc.sync.dma_start(ss_t[:], ssT[rows, :])
                ssr = wstream.tile([128, OUT_F], BF16, tag="ssr")
                nc.vector.tensor_scalar_mul(ssr[:], ss_t[:], 1.0 / 6.0)
                bw_t = wstream.tile([128, OUT_F], F32, tag="bw")
                nc.sync.dma_start(bw_t[:], bwT[rows, :])
                nc.vector.tensor_copy(bw_bf[:, c], bw_t[:])
                for k in range(NK):
                    sw_t = wstream.tile([128, OUT_F], BF16, tag="sw", bufs=3)
                    nc.sync.dma_start(sw_t[:], swT[k, rows, :])
                    nc.vector.tensor_tensor(
                        sw_bf[:, k, c], sw_t[:], ssr[:], op=mybir.AluOpType.mult)

            wstream_cm.__exit__(None, None, None)

            # ---- per batch-quarter ----
            for q in range(NQ):
                bsl = slice(q * QB, (q + 1) * QB)
                x_t = xload.tile([128, NC_IN, QB], F32)
                nc.sync.dma_start(
                    x_t[:], xT[:, bsl].rearrange("(c p) b -> p c b", p=128))
                psl = []
                for mb in range(QB // 128):
                    for no in range(OUT_F // 512):
                        ps = psum.tile([128, 512], F32, name=f"ps{mb}{no}", tag=f"ps{mb}{no}")
                        psl.append((mb, no, ps))
                for c in range(NC_IN):
                    xa = x_t[:, c]
                    silu_bf = qdata.tile([128, QB], mybir.dt.float32r, tag="silu", name="silu_bf")
                    bases = qdata.tile([128, NK, QB], BF16, tag="bases", name="bases")
                    nc.scalar.activation(silu_bf[:], xa, AF.Silu)
                    ge1 = tmp.tile([128, QB], BF16, tag="ge1")
                    nc.vector.tensor_scalar(ge1[:], xa, G4, None, op0=mybir.AluOpType.is_ge)
                    ge2 = tmp.tile([128, QB], BF16, tag="ge2")
                    nc.vector.tensor_scalar(ge2[:], xa, G5, None, op0=mybir.AluOpType.is_ge)
                    m_bf = tmp.tile([128, QB], BF16, tag="m")
                    nc.vector.tensor_tensor(m_bf[:], ge1[:], ge2[:], op=mybir.AluOpType.add)
                    u0 = tmp.tile([128, QB], F32, tag="u0")   # u = 1.5x+1.5
                    nc.scalar.activation(u0[:], xa, AF.Identity, bias=1.5, scale=1.5)
                    t = tmp.tile([128, QB], BF16, tag="t")
                    nc.vector.scalar_tensor_tensor(
                        t[:], m_bf[:], -1.0, u0[:],
                        op0=mybir.AluOpType.mult, op1=mybir.AluOpType.add)
                    t2 = tmp.tile([128, QB], BF16, tag="t2")
                    nc.vector.tensor_tensor(t2[:], t[:], t[:], op=mybir.AluOpType.mult)
                    t3 = tmp.tile([128, QB], BF16, tag="t3")
                    nc.vector.tensor_tensor(t3[:], t2[:], t[:], op=mybir.AluOpType.mult)
                    v = tmp.tile([128, QB], BF16, tag="v")
                    nc.scalar.activation(v[:], t[:], AF.Identity, bias=1.0, scale=-1.0)
                    v2 = tmp.tile([128, QB], BF16, tag="v2")
                    nc.vector.tensor_tensor(v2[:], v[:], v[:], op=mybir.AluOpType.mult)
                    S0 = tmp.tile([128, QB], BF16, tag="S0")
                    nc.vector.tensor_tensor(S0[:], v2[:], v[:], op=mybir.AluOpType.mult)
                    w1 = tmp.tile([128, QB], BF16, tag="w1")
                    nc.scalar.activation(w1[:], t[:], AF.Identity, bias=-6.0, scale=3.0)
                    S1s = tmp.tile([128, QB], BF16, tag="S1s")
                    nc.vector.tensor_tensor(S1s[:], w1[:], t2[:], op=mybir.AluOpType.mult)
                    S1 = tmp.tile([128, QB], BF16, tag="S1")
                    nc.scalar.activation(S1[:], S1s[:], AF.Identity, bias=4.0, scale=1.0)
                    sum1 = tmp.tile([128, QB], BF16, tag="sum1")
                    nc.vector.tensor_tensor(sum1[:], S0[:], S1s[:], op=mybir.AluOpType.add)
                    sum2 = tmp.tile([128, QB], BF16, tag="sum2")
                    nc.vector.tensor_tensor(sum2[:], sum1[:], t3[:], op=mybir.AluOpType.add)
                    S2 = tmp.tile([128, QB], BF16, tag="S2")
                    nc.scalar.activation(S2[:], sum2[:], AF.Identity, bias=2.0, scale=-1.0)
                    is0 = tmp.tile([128, QB], BF16, tag="is0")
                    nc.scalar.activation(is0[:], ge1[:], AF.Identity, bias=1.0, scale=-1.0)
                    is1 = tmp.tile([128, QB], BF16, tag="is1")
                    nc.vector.tensor_tensor(is1[:], ge1[:], ge2[:], op=mybir.AluOpType.subtract)
                    is2 = ge2
                    S3 = t3
                    MUL, ADD = mybir.AluOpType.mult, mybir.AluOpType.add
                    TT = nc.vector.tensor_tensor

                    def mk(tag):
                        return tmp.tile([128, QB], BF16, tag=tag, name=tag)
                    TT(bases[:, 0], is0[:], S0[:], op=MUL)
                    a = mk("pa")
                    b = mk("pb")
                    TT(a[:], is0[:], S1[:], op=MUL)
                    TT(b[:], is1[:], S0[:], op=MUL)
                    TT(bases[:, 1], a[:], b[:], op=ADD)
                    a2 = mk("pc")
                    b2 = mk("pd")
                    d2 = mk("pe")
                    TT(a2[:], is0[:], S2[:], op=MUL)
                    TT(b2[:], is1[:], S1[:], op=MUL)
                    TT(d2[:], is2[:], S0[:], op=MUL)
                    e2 = mk("pf")
                    TT(e2[:], a2[:], b2[:], op=ADD)
                    TT(bases[:, 2], e2[:], d2[:], op=ADD)
                    a3 = mk("pg")
                    b3 = mk("ph")
                    d3 = mk("pi")
                    TT(a3[:], is0[:], S3[:], op=MUL)
                    TT(b3[:], is1[:], S2[:], op=MUL)
                    TT(d3[:], is2[:], S1[:], op=MUL)
                    e3 = mk("pj")
                    TT(e3[:], a3[:], b3[:], op=ADD)
                    TT(bases[:, 3], e3[:], d3[:], op=ADD)
                    a4 = mk("pk")
                    b4 = mk("pl")
                    TT(a4[:], is1[:], S3[:], op=MUL)
                    TT(b4[:], is2[:], S2[:], op=MUL)
                    TT(bases[:, 4], a4[:], b4[:], op=ADD)
                    TT(bases[:, 5], is2[:], S3[:], op=MUL)

                    for mb, no, ps in psl:
                        brow = slice(mb * 128, (mb + 1) * 128)
                        ocol = slice(no * 512, (no + 1) * 512)
                        for k in range(NK):
                            nc.tensor.matmul(
                                ps[:], bases[:, k, brow], sw_bf[:, k, c, ocol],
                                start=(c == 0 and k == 0), stop=False)
                        nc.tensor.matmul(
                            ps[:], silu_bf[:, brow], bw_bf[:, c, ocol],
                            start=False, stop=(c == NC_IN - 1))

                for mb, no, ps in psl:
                    ocol = slice(no * 512, (no + 1) * 512)
                    st = stage.tile([128, 512], F32, name="st", tag="st")
                    nc.scalar.copy(st[:], ps[:])
                    nc.sync.dma_start(
                        Y[q * QB + mb * 128: q * QB + (mb + 1) * 128, ocol], st[:])
    _split_multi_waits(nc)
    return nc


_NC_CACHE = None


def kernel(x, base_weight, spline_weight, spline_scaler, grid):
    global _NC_CACHE
    if _NC_CACHE is None:
        _NC_CACHE = _build()
    nc = _NC_CACHE
    import ml_dtypes
    swT = np.ascontiguousarray(spline_weight.transpose(2, 1, 0)).astype(ml_dtypes.bfloat16)
    ssT = np.ascontiguousarray(spline_scaler.T).astype(np.float32)
    bwT = np.ascontiguousarray(base_weight.T).astype(np.float32)
    in_maps = []
    for c in range(N_CORES):
        xs = x[c * B_CORE:(c + 1) * B_CORE]
        in_maps.append({
            "xT": np.ascontiguousarray(xs.T).astype(np.float32),
            "swT": swT, "ssT": ssT, "bwT": bwT,
        })
    res = run_bass_kernel_spmd(nc, in_maps, core_ids=list(range(N_CORES)))
    return np.concatenate([res.results[c]["Y"] for c in range(N_CORES)], axis=0)
